# revision 14
# baseline (speedup 1.0000x reference)
"""Trainium2 Bass kernel: multi-head encoder-decoder attention.

nn_MultiHeadEncDecAttention — B=1, N=4096, d_model=768, 12 heads, d_k=64.

Self-contained harness entry point: `kernel(**inputs) -> np.ndarray` takes
the FULL unsharded inputs (as produced by the problem's setup_inputs()),
distributes the work across 8 NeuronCores (heads x query-range sharding),
runs a Bass/Tile SPMD program, and reassembles the full output.

Sharding: core pair p in {0..3} owns heads {3p, 3p+1, 3p+2}; within a
pair, core 2p handles query rows [0, 2048) and core 2p+1 rows [2048, 4096).
Each core computes its heads' attention for its query rows plus the partial
output projection for those heads; the host sums the 4 head-partials per
query half (b_o and the b_v contribution are folded in on the host, which
is exact because softmax rows sum to 1).

Optimizations vs the original baseline (measured ~2x faster device body):
  - all tensors bf16 (host-converted), halving HBM traffic
  - host pre-tiles every DRAM operand into SBUF layout so each DMA is one
    contiguous run per partition (~128 descriptors instead of ~768), and the
    SP DMA queue carries only ~27 transfers total (no SBUF-SBUF copies)
  - SBUF half-duplications done as DVE partition-shifted copies instead of
    DMAs (keeps the serial DMA dispatch queue free for input streaming)
  - output projection packs heads 0+1 into one 128-contraction matmul
    (w_o rows 0:128 as one stationary tile)
  - yT returned as bf16 in store-friendly tiling, host accumulates in f32
  - exp pool 6 buffers to decouple ACT exp production from PE consumption
  (gpsimd/SWDGE deliberately unused: any gpsimd instruction adds ~110us of
  fixed per-execution runtime cost; prologue interleaving of q-block-0
  scores into the K/V projection was tried and measured slower on HW)
"""

import sys

sys.path.insert(0, "/opt/trn_rl_repo")

from contextlib import ExitStack

import numpy as np
import ml_dtypes

import concourse.tile as tile
from concourse import bacc, mybir
from concourse.bass_utils import run_bass_kernel_spmd

F32 = mybir.dt.float32
F32R = mybir.dt.float32r
BF16 = mybir.dt.bfloat16

D = 768          # d_model
DK = 64          # per-head dim
HPC = 3          # heads per core
P = 128          # SBUF partitions
QB = 512         # matmul moving-dim block
DT = D // P      # contraction k-tiles over d_model
N_CORES = 8


def build_program(NQ=2048, NK=4096, kgroup=2, repeat=1):
    """Build + compile the per-core SPMD program (identical on all cores).

    Layout strategy (everything "transposed"; host passes x^T / enc^T tiled):
      QT[h] [128, NQ], KT[h] [128, NK] — data in both partition halves so
                                         scores matmuls can PE-row-tile
      V[h]  [128, 32, 96]              — natural layout via bf16 DMA
                                         transpose; column 64 = ones
      scoresT = KT-tile.T @ QT-block  -> PSUM [128(kpos), 512(q)]
      expT    = exp(0.125*scoresT)    -> SBUF bf16 (ACT, scale folded in)
      AV      = [V|1].T @ expT        -> PSUM [65, 512]; row 64 = denom
      yT     += w_o-slice.T @ (AV[0:64] * recip(AV[64]))  over heads
    """
    KT_N = NK // P           # kpos tiles
    QBS = NQ // QB           # q blocks
    KB_N = NK // QB          # kpos blocks for the K/V projection
    EXP_TILES = 12 if KT_N >= 12 else KT_N   # kpos-tiles per exp buffer

    nc = bacc.Bacc("TRN2", target_bir_lowering=False, debug=False)

    # host-pretiled DRAM operands: one contiguous run per partition line
    xq = nc.dram_tensor("xq", [QBS, P, DT, QB], BF16, kind="ExternalInput").ap()
    ek = nc.dram_tensor("ek", [KB_N, P, DT, QB], BF16, kind="ExternalInput").ap()
    wq_d = nc.dram_tensor("wq_d", [P, DT, HPC * DK], BF16, kind="ExternalInput").ap()
    # wkv = concat([w_k cols, w_v cols]); projection passes use 128-col
    # slices: [wk0|wk1], [wk2|wv0], [wv1|wv2]
    wkv_d = nc.dram_tensor("wkv_d", [P, DT, 2 * HPC * DK], BF16, kind="ExternalInput").ap()
    woA_d = nc.dram_tensor("woA_d", [P, D], BF16, kind="ExternalInput").ap()  # heads 0,1
    woB_d = nc.dram_tensor("woB_d", [DK, D], BF16, kind="ExternalInput").ap()  # head 2
    bq = nc.dram_tensor("bq", [HPC * DK, 1], F32, kind="ExternalInput").ap()
    bk = nc.dram_tensor("bk", [HPC * DK, 1], F32, kind="ExternalInput").ap()
    yT = nc.dram_tensor("yT", [QBS, P, DT, QB], BF16, kind="ExternalOutput").ap()

    with tile.TileContext(nc) as tc, ExitStack() as ctx:
        consts = ctx.enter_context(tc.tile_pool(name="consts", bufs=1))
        persist = ctx.enter_context(tc.tile_pool(name="persist", bufs=1))
        stream = ctx.enter_context(tc.tile_pool(name="stream", bufs=3))
        small = ctx.enter_context(tc.tile_pool(name="small", bufs=2))
        ysb_pool = ctx.enter_context(tc.tile_pool(name="ysb", bufs=2))
        exp_pool = ctx.enter_context(tc.tile_pool(name="exp", bufs=6))
        ps_s = ctx.enter_context(tc.tile_pool(name="ps_s", bufs=2, space="PSUM"))
        ps_mm = ctx.enter_context(tc.tile_pool(name="ps_mm", bufs=2, space="PSUM"))

        for _rep in range(repeat):
            # ---- constants -------------------------------------------------
            wkv_sb = consts.tile([P, DT, 2 * HPC * DK], BF16)
            nc.sync.dma_start(out=wkv_sb, in_=wkv_d)
            wq_sb = consts.tile([P, DT, HPC * DK], BF16)
            nc.sync.dma_start(out=wq_sb, in_=wq_d)
            woA_sb = consts.tile([P, D], BF16)
            nc.sync.dma_start(out=woA_sb, in_=woA_d)
            woB_sb = consts.tile([DK, D], BF16)
            nc.sync.dma_start(out=woB_sb, in_=woB_d)
            bqA = consts.tile([P, 1], F32)
            nc.sync.dma_start(out=bqA, in_=bq[0:P, :])
            bqB = consts.tile([DK, 1], F32)
            nc.sync.dma_start(out=bqB, in_=bq[P : P + DK, :])
            bkA = consts.tile([P, 1], F32)
            nc.sync.dma_start(out=bkA, in_=bk[0:P, :])
            bkB = consts.tile([DK, 1], F32)
            nc.sync.dma_start(out=bkB, in_=bk[P : P + DK, :])
            ones1_f = consts.tile([P, DK], F32)
            nc.vector.memset(ones1_f[DK : DK + 1, :], 1.0)
            ones1 = ones1_f.bitcast(F32R)

            # ---- persistent per-head tensors ------------------------------
            kT = [persist.tile([P, NK], BF16, name=f"kT{h}") for h in range(HPC)]
            qT = [persist.tile([P, NQ], BF16, name=f"qT{h}") for h in range(HPC)]
            # V row stride padded to 96 els (192 B): keeps each DMA-transpose
            # dest 32-byte aligned (XBAR); col 64 = ones, cols 65..95 zero
            v = [persist.tile([P, KT_N, 96], BF16, name=f"v{h}") for h in range(HPC)]
            # vt shares the exp pool tag: dead after the V transposes, so its
            # slots are recycled as exp buffers during attention
            vt = [exp_pool.tile([P, NK], BF16, name=f"vt{h}", tag="e") for h in range(HPC)]
            for h in range(HPC):
                nc.vector.memset(v[h][:, :, DK : DK + 1], 1.0)
                nc.vector.memset(v[h][:, :, DK + 1 : 96], 0.0)

            # ---- Q projection (DVE bias-adds + partition-shifted dups) ----
            def emit_qproj(qb):
                qs = slice(qb * QB, (qb + 1) * QB)
                x_t = stream.tile([P, DT, QB], BF16, name="x_t", tag="enc")
                nc.sync.dma_start(out=x_t, in_=xq[qb])
                ps = ps_mm.tile([P, QB], F32, tag="mm", name="ps_q01")
                for t in range(DT):
                    nc.tensor.matmul(
                        ps, wq_sb[:, t, 0:P], x_t[:, t, :],
                        start=(t == 0), stop=(t == DT - 1),
                    )
                nc.vector.tensor_scalar_add(
                    out=qT[0][0:DK, qs], in0=ps[0:DK], scalar1=bqA[0:DK]
                )
                nc.vector.tensor_scalar_add(
                    out=qT[1][DK:P, qs], in0=ps[DK:P], scalar1=bqA[DK:P]
                )
                ps2 = ps_mm.tile([P, QB], F32, tag="mm", name="ps_q2")
                for t in range(DT):
                    nc.tensor.matmul(
                        ps2[0:DK], wq_sb[:, t, P : P + DK], x_t[:, t, :],
                        start=(t == 0), stop=(t == DT - 1),
                    )
                nc.vector.tensor_scalar_add(
                    out=qT[2][0:DK, qs], in0=ps2[0:DK], scalar1=bqB[0:DK]
                )
                nc.vector.tensor_copy(out=qT[0][DK:P, qs], in_=qT[0][0:DK, qs])
                nc.vector.tensor_copy(out=qT[1][0:DK, qs], in_=qT[1][DK:P, qs])
                nc.vector.tensor_copy(out=qT[2][DK:P, qs], in_=qT[2][0:DK, qs])

            emit_qproj(0)

            # ---- K/V projection -------------------------------------------
            for kb in range(KB_N):
                ks = slice(kb * QB, (kb + 1) * QB)
                enc_t = stream.tile([P, DT, QB], BF16, name="enc_t", tag="enc")
                nc.sync.dma_start(out=enc_t, in_=ek[kb])
                for pi in range(3):
                    ps = ps_mm.tile([P, QB], F32, tag="mm", name="ps_kv")
                    for t in range(DT):
                        nc.tensor.matmul(
                            ps, wkv_sb[:, t, pi * P : (pi + 1) * P],
                            enc_t[:, t, :], start=(t == 0), stop=(t == DT - 1),
                        )
                    if pi == 0:
                        nc.vector.tensor_scalar_add(
                            out=kT[0][0:DK, ks], in0=ps[0:DK], scalar1=bkA[0:DK]
                        )
                        nc.vector.tensor_scalar_add(
                            out=kT[1][DK:P, ks], in0=ps[DK:P], scalar1=bkA[DK:P]
                        )
                        nc.vector.tensor_copy(out=kT[0][DK:P, ks], in_=kT[0][0:DK, ks])
                        nc.vector.tensor_copy(out=kT[1][0:DK, ks], in_=kT[1][DK:P, ks])
                    elif pi == 1:
                        nc.vector.tensor_scalar_add(
                            out=kT[2][0:DK, ks], in0=ps[0:DK], scalar1=bkB[0:DK]
                        )
                        nc.vector.tensor_copy(out=kT[2][DK:P, ks], in_=kT[2][0:DK, ks])
                        nc.vector.tensor_copy(out=vt[0][DK:P, ks], in_=ps[DK:P])
                    else:
                        nc.vector.tensor_copy(out=vt[1][0:DK, ks], in_=ps[0:DK])
                        nc.vector.tensor_copy(out=vt[2][DK:P, ks], in_=ps[DK:P])

            # ---- V: bf16 DMA transpose into natural layout ----------------
            for h in range(HPC):
                src_rows = slice(DK, P) if h != 1 else slice(0, DK)
                nc.sync.dma_start(
                    out=v[h][:, :, 0:DK], in_=vt[h][src_rows, :], transpose=True
                )

            for _qb in range(1, QBS):
                emit_qproj(_qb)

            # ---- attention + output projection ----------------------------
            # Software pipeline across (qb, h) units: emit scores+exp for unit
            # i, then AV+normalize for unit i-1 (whose exp overlapped unit i's
            # scores on ACT), then the output projection for a q block once its
            # last head is normalized.
            NGRP = (KT_N + kgroup - 1) // kgroup
            units = [(qb, h) for qb in range(QBS) for h in range(HPC)]
            oT = {}

            def emit_scores_exp(qb, h):
                qs = slice(qb * QB, (qb + 1) * QB)
                exp_bufs = []
                for g in range(NGRP):
                    gsz = min(kgroup, KT_N - g * kgroup)
                    ps = ps_s.tile([P, kgroup * QB], F32, tag="s", name="ps_sc")
                    for j in range(gsz):
                        kt = g * kgroup + j
                        half = slice(0, DK) if kt % 2 == 0 else slice(DK, P)
                        tp = (0, 0) if kt % 2 == 0 else (DK, 0)
                        nc.tensor.matmul(
                            ps[:, j * QB : (j + 1) * QB],
                            kT[h][half, kt * P : (kt + 1) * P],
                            qT[h][half, qs],
                            start=True,
                            stop=True,
                            tile_position=tp,
                        )
                    if (g * kgroup) % EXP_TILES == 0:
                        eb = exp_pool.tile([P, EXP_TILES * QB], BF16, tag="e", name="expT")
                        exp_bufs.append(eb)
                    off = (g * kgroup) % EXP_TILES
                    nc.scalar.activation(
                        out=exp_bufs[-1][:, off * QB : (off + gsz) * QB],
                        in_=ps[:, 0 : gsz * QB],
                        func=mybir.ActivationFunctionType.Exp,
                        scale=0.125,
                    )
                return exp_bufs

            def emit_av_norm(qb, h, exp_bufs):
                av = ps_mm.tile([P, QB], F32, tag="mm", name="ps_av_t")
                for kt in range(KT_N):
                    eb = exp_bufs[kt // EXP_TILES]
                    off = kt % EXP_TILES
                    nc.tensor.matmul(
                        av[0 : DK + 1],
                        v[h][:, kt, 0 : DK + 1],
                        eb[:, off * QB : (off + 1) * QB],
                        start=(kt == 0),
                        stop=(kt == KT_N - 1),
                    )
                rt = small.tile([P, QB], F32R, tag="rt", name="recip_t")
                with nc.allow_low_precision(reason="f32r recip feeds f32r matmul"):
                    nc.vector.reciprocal(out=rt[DK : DK + 1], in_=av[DK : DK + 1])
                # broadcast the reciprocal row across partitions 0..63 with a
                # K=1 PE matmul against a ones column (row group 64)
                rb = ps_mm.tile([P, QB], F32, tag="mmo", name="ps_rb")
                nc.tensor.matmul(
                    rb[0:DK],
                    ones1[DK : DK + 1, :],
                    rt[DK : DK + 1, :],
                    start=True,
                    stop=True,
                    tile_position=(DK, 0),
                )
                rbs = small.tile([DK, QB], F32, tag="rbs", name="rb_sb")
                nc.vector.tensor_copy(out=rbs, in_=rb[0:DK])
                # heads 0,1 pack into one [128, QB] tile for a K=128 out-proj
                if h < 2:
                    if h == 0:
                        o01 = small.tile([P, QB], BF16, tag="o01", name="o01_t")
                        oT[(qb, "01")] = o01
                    dst = oT[(qb, "01")][h * DK : (h + 1) * DK, :]
                else:
                    o2 = small.tile([DK, QB], BF16, tag="o2", name="o2_t")
                    oT[(qb, "2")] = o2
                    dst = o2
                nc.vector.tensor_mul(out=dst, in0=av[0:DK], in1=rbs)

            def emit_outproj(qb):
                ysb = ysb_pool.tile([P, DT, QB], BF16, tag="y", name="y_t")
                for dt_i in range(DT):
                    pso = ps_mm.tile([P, QB], F32, tag="mmo", name="ps_o")
                    nc.tensor.matmul(
                        pso,
                        woA_sb[:, dt_i * P : (dt_i + 1) * P],
                        oT[(qb, "01")],
                        start=True,
                        stop=False,
                    )
                    nc.tensor.matmul(
                        pso,
                        woB_sb[:, dt_i * P : (dt_i + 1) * P],
                        oT[(qb, "2")],
                        start=False,
                        stop=True,
                    )
                    nc.vector.tensor_copy(out=ysb[:, dt_i, :], in_=pso)
                nc.sync.dma_start(out=yT[qb], in_=ysb)

            pend = None  # (qb, h, exp_bufs) of the previous unit
            for qb, h in units:
                ebs = emit_scores_exp(qb, h)
                if pend is not None:
                    pqb, ph, pebs = pend
                    emit_av_norm(pqb, ph, pebs)
                    if ph == HPC - 1:
                        emit_outproj(pqb)
                pend = (qb, h, ebs)
            pqb, ph, pebs = pend
            emit_av_norm(pqb, ph, pebs)
            emit_outproj(pqb)

    nc.compile()
    return nc


def _tile_T(mat_T, nblk):
    """[D, N] f32 -> [nblk, 128, DT, 512] bf16 (pre-tiled for SBUF)."""
    Dd, N = mat_T.shape
    a = mat_T.reshape(DT, P, nblk, QB).transpose(2, 1, 0, 3)
    return np.ascontiguousarray(a.astype(ml_dtypes.bfloat16))


def shard_inputs(x, encoding, w_q, b_q, w_k, b_k, w_v, b_v, w_o, b_o):
    """Full inputs -> list of 8 per-core input dicts (numpy, contiguous)."""
    N = x.shape[1]
    xT_full = np.asarray(x, np.float32)[0].T                      # [D, N]
    encT = np.asarray(encoding, np.float32)[0].T                  # [D, N]
    w_q, w_k, w_v, w_o = (np.asarray(a, np.float32) for a in (w_q, w_k, w_v, w_o))
    b_q, b_k = np.asarray(b_q, np.float32), np.asarray(b_k, np.float32)
    ek_t = _tile_T(encT, N // QB)
    in_maps = []
    for core in range(N_CORES):
        p = core // 2
        hsel = slice(HPC * p * DK, HPC * (p + 1) * DK)
        qsel = slice(0, N // 2) if core % 2 == 0 else slice(N // 2, N)
        wo_h = w_o[hsel, :]
        in_maps.append(
            {
                "xq": _tile_T(xT_full[:, qsel], (N // 2) // QB),
                "ek": ek_t,
                "wkv_d": np.ascontiguousarray(
                    np.concatenate([w_k[:, hsel], w_v[:, hsel]], axis=1)
                    .reshape(DT, P, 2 * HPC * DK)
                    .astype(ml_dtypes.bfloat16)
                    .transpose(1, 0, 2)
                ),
                "wq_d": np.ascontiguousarray(
                    w_q[:, hsel]
                    .reshape(DT, P, HPC * DK)
                    .astype(ml_dtypes.bfloat16)
                    .transpose(1, 0, 2)
                ),
                "woA_d": np.ascontiguousarray(wo_h[0:P].astype(ml_dtypes.bfloat16)),
                "woB_d": np.ascontiguousarray(wo_h[P:].astype(ml_dtypes.bfloat16)),
                "bq": np.ascontiguousarray(b_q[hsel].reshape(-1, 1)),
                "bk": np.ascontiguousarray(b_k[hsel].reshape(-1, 1)),
            }
        )
    return in_maps


def combine_outputs(results, b_v, w_o, b_o, N, dtype):
    """Per-core yT partials -> full [1, N, D] output (host-side biases)."""
    half = N // 2
    nblk = half // QB
    y = np.zeros((N, D), np.float32)
    for core, res in enumerate(results):
        yt = np.asarray(res["yT"]).astype(np.float32)  # [nblk, 128, DT, 512]
        # [nblk, P, DT, QB] -> [D, half]: d = dt*P + p, n = qb*QB + q
        part = yt.transpose(2, 1, 0, 3).reshape(D, half).T
        if core % 2 == 0:
            y[:half] += part
        else:
            y[half:] += part
    y += np.asarray(b_v, np.float32) @ np.asarray(w_o, np.float32) + np.asarray(
        b_o, np.float32
    )
    return np.ascontiguousarray(y[None]).astype(dtype)


_PROGRAM_CACHE = {}


def _get_program():
    key = "main"
    if key not in _PROGRAM_CACHE:
        _PROGRAM_CACHE[key] = build_program()
    return _PROGRAM_CACHE[key]


def kernel(x, encoding, w_q, b_q, w_k, b_k, w_v, b_v, w_o, b_o):
    nc = _get_program()
    in_maps = shard_inputs(x, encoding, w_q, b_q, w_k, b_k, w_v, b_v, w_o, b_o)
    res = run_bass_kernel_spmd(nc, in_maps, core_ids=list(range(N_CORES)))
    return combine_outputs(
        res.results, b_v, w_o, b_o, np.asarray(x).shape[1], np.asarray(x).dtype
    )


# revision 16
# speedup vs baseline: 1.0389x; 1.0389x over previous
"""Trainium2 Bass kernel: multi-head encoder-decoder attention.

nn_MultiHeadEncDecAttention — B=1, N=4096, d_model=768, 12 heads, d_k=64.

Self-contained harness entry point: `kernel(**inputs) -> np.ndarray` takes
the FULL unsharded inputs (as produced by the problem's setup_inputs()),
distributes the work across 8 NeuronCores (heads x query-range sharding),
runs a Bass/Tile SPMD program, and reassembles the full output.

Sharding: core pair p in {0..3} owns heads {3p, 3p+1, 3p+2}; within a
pair, core 2p handles query rows [0, 2048) and core 2p+1 rows [2048, 4096).
Each core computes its heads' attention for its query rows plus the partial
output projection for those heads; the host sums the 4 head-partials per
query half (b_o and the b_v contribution are folded in on the host, which
is exact because softmax rows sum to 1).

Optimizations vs the original baseline (measured ~2x faster device body):
  - all tensors bf16 (host-converted), halving HBM traffic
  - host pre-tiles every DRAM operand into SBUF layout so each DMA is one
    contiguous run per partition (~128 descriptors instead of ~768), and the
    SP DMA queue carries only ~27 transfers total (no SBUF-SBUF copies)
  - SBUF half-duplications done as DVE partition-shifted copies instead of
    DMAs (keeps the serial DMA dispatch queue free for input streaming)
  - output projection packs heads 0+1 into one 128-contraction matmul
    (w_o rows 0:128 as one stationary tile)
  - yT returned as bf16 in store-friendly tiling, host accumulates in f32
  - exp pool 6 buffers to decouple ACT exp production from PE consumption
  (gpsimd/SWDGE deliberately unused: any gpsimd instruction adds ~110us of
  fixed per-execution runtime cost; prologue interleaving of q-block-0
  scores into the K/V projection was tried and measured slower on HW)
"""

import sys

sys.path.insert(0, "/opt/trn_rl_repo")

from contextlib import ExitStack

import numpy as np
import ml_dtypes

import concourse.tile as tile
from concourse import bacc, mybir
from concourse.bass_utils import run_bass_kernel_spmd

F32 = mybir.dt.float32
F32R = mybir.dt.float32r
BF16 = mybir.dt.bfloat16

D = 768          # d_model
DK = 64          # per-head dim
HPC = 3          # heads per core
P = 128          # SBUF partitions
QB = 512         # matmul moving-dim block
DT = D // P      # contraction k-tiles over d_model
N_CORES = 8


def build_program(NQ=2048, NK=4096, kgroup=2, repeat=1):
    """Build + compile the per-core SPMD program (identical on all cores).

    Layout strategy (everything "transposed"; host passes x^T / enc^T tiled):
      QT[h] [128, NQ], KT[h] [128, NK] — data in both partition halves so
                                         scores matmuls can PE-row-tile
      V[h]  [128, 32, 96]              — natural layout via bf16 DMA
                                         transpose; column 64 = ones
      scoresT = KT-tile.T @ QT-block  -> PSUM [128(kpos), 512(q)]
      expT    = exp(0.125*scoresT)    -> SBUF bf16 (ACT, scale folded in)
      AV      = [V|1].T @ expT        -> PSUM [65, 512]; row 64 = denom
      yT     += w_o-slice.T @ (AV[0:64] * recip(AV[64]))  over heads
    """
    KT_N = NK // P           # kpos tiles
    QBS = NQ // QB           # q blocks
    KB_N = NK // QB          # kpos blocks for the K/V projection
    EXP_TILES = 12 if KT_N >= 12 else KT_N   # kpos-tiles per exp buffer

    nc = bacc.Bacc("TRN2", target_bir_lowering=False, debug=False)

    # host-pretiled DRAM operands: one contiguous run per partition line
    xq = nc.dram_tensor("xq", [QBS, P, DT, QB], BF16, kind="ExternalInput").ap()
    ek = nc.dram_tensor("ek", [KB_N, P, DT, QB], BF16, kind="ExternalInput").ap()
    wq_d = nc.dram_tensor("wq_d", [P, DT, HPC * DK], BF16, kind="ExternalInput").ap()
    # wkv = concat([w_k cols, w_v cols]); projection passes use 128-col
    # slices: [wk0|wk1], [wk2|wv0], [wv1|wv2]
    wkv_d = nc.dram_tensor("wkv_d", [P, DT, 2 * HPC * DK], BF16, kind="ExternalInput").ap()
    woA_d = nc.dram_tensor("woA_d", [P, D], BF16, kind="ExternalInput").ap()  # heads 0,1
    woB_d = nc.dram_tensor("woB_d", [DK, D], BF16, kind="ExternalInput").ap()  # head 2
    bq = nc.dram_tensor("bq", [HPC * DK, 1], F32, kind="ExternalInput").ap()
    bk = nc.dram_tensor("bk", [HPC * DK, 1], F32, kind="ExternalInput").ap()
    yT = nc.dram_tensor("yT", [QBS, P, DT, QB], BF16, kind="ExternalOutput").ap()

    with tile.TileContext(nc) as tc, ExitStack() as ctx:
        consts = ctx.enter_context(tc.tile_pool(name="consts", bufs=1))
        persist = ctx.enter_context(tc.tile_pool(name="persist", bufs=1))
        stream = ctx.enter_context(tc.tile_pool(name="stream", bufs=3))
        small = ctx.enter_context(tc.tile_pool(name="small", bufs=2))
        ysb_pool = ctx.enter_context(tc.tile_pool(name="ysb", bufs=2))
        exp_pool = ctx.enter_context(tc.tile_pool(name="exp", bufs=6))
        ps_s = ctx.enter_context(tc.tile_pool(name="ps_s", bufs=2, space="PSUM"))
        ps_mm = ctx.enter_context(tc.tile_pool(name="ps_mm", bufs=2, space="PSUM"))

        for _rep in range(repeat):
            # ---- constants -------------------------------------------------
            wkv_sb = consts.tile([P, DT, 2 * HPC * DK], BF16)
            nc.sync.dma_start(out=wkv_sb, in_=wkv_d)
            wq_sb = consts.tile([P, DT, HPC * DK], BF16)
            nc.sync.dma_start(out=wq_sb, in_=wq_d)
            woA_sb = consts.tile([P, D], BF16)
            nc.sync.dma_start(out=woA_sb, in_=woA_d)
            woB_sb = consts.tile([DK, D], BF16)
            nc.sync.dma_start(out=woB_sb, in_=woB_d)
            bqA = consts.tile([P, 1], F32)
            nc.sync.dma_start(out=bqA, in_=bq[0:P, :])
            bqB = consts.tile([DK, 1], F32)
            nc.sync.dma_start(out=bqB, in_=bq[P : P + DK, :])
            bkA = consts.tile([P, 1], F32)
            nc.sync.dma_start(out=bkA, in_=bk[0:P, :])
            bkB = consts.tile([DK, 1], F32)
            nc.sync.dma_start(out=bkB, in_=bk[P : P + DK, :])
            ones1_f = consts.tile([P, DK], F32)
            nc.vector.memset(ones1_f[DK : DK + 1, :], 1.0)
            ones1 = ones1_f.bitcast(F32R)

            # ---- persistent per-head tensors ------------------------------
            kT = [persist.tile([P, NK], BF16, name=f"kT{h}") for h in range(HPC)]
            qT = [persist.tile([P, NQ], BF16, name=f"qT{h}") for h in range(HPC)]
            # V row stride padded to 96 els (192 B): keeps each DMA-transpose
            # dest 32-byte aligned (XBAR); col 64 = ones, cols 65..95 zero
            v = [persist.tile([P, KT_N, 96], BF16, name=f"v{h}") for h in range(HPC)]
            # vt shares the exp pool tag: dead after the V transposes, so its
            # slots are recycled as exp buffers during attention
            vt = [exp_pool.tile([P, NK], BF16, name=f"vt{h}", tag="e") for h in range(HPC)]
            for h in range(HPC):
                nc.vector.memset(v[h][:, :, DK : DK + 1], 1.0)
                nc.vector.memset(v[h][:, :, DK + 1 : 96], 0.0)

            # ---- Q projection (DVE bias-adds + partition-shifted dups) ----
            def emit_qproj(qb):
                qs = slice(qb * QB, (qb + 1) * QB)
                x_t = stream.tile([P, DT, QB], BF16, name="x_t", tag="enc")
                nc.sync.dma_start(out=x_t, in_=xq[qb])
                ps = ps_mm.tile([P, QB], F32, tag="mm", name="ps_q01")
                for t in range(DT):
                    nc.tensor.matmul(
                        ps, wq_sb[:, t, 0:P], x_t[:, t, :],
                        start=(t == 0), stop=(t == DT - 1),
                    )
                nc.vector.tensor_scalar_add(
                    out=qT[0][0:DK, qs], in0=ps[0:DK], scalar1=bqA[0:DK]
                )
                nc.vector.tensor_scalar_add(
                    out=qT[1][DK:P, qs], in0=ps[DK:P], scalar1=bqA[DK:P]
                )
                ps2 = ps_mm.tile([P, QB], F32, tag="mm", name="ps_q2")
                for t in range(DT):
                    nc.tensor.matmul(
                        ps2[0:DK], wq_sb[:, t, P : P + DK], x_t[:, t, :],
                        start=(t == 0), stop=(t == DT - 1),
                    )
                nc.vector.tensor_scalar_add(
                    out=qT[2][0:DK, qs], in0=ps2[0:DK], scalar1=bqB[0:DK]
                )
                nc.vector.tensor_copy(out=qT[0][DK:P, qs], in_=qT[0][0:DK, qs])
                nc.vector.tensor_copy(out=qT[1][0:DK, qs], in_=qT[1][DK:P, qs])
                nc.vector.tensor_copy(out=qT[2][DK:P, qs], in_=qT[2][0:DK, qs])

            emit_qproj(0)

            # ---- K/V projection -------------------------------------------
            for kb in range(KB_N):
                ks = slice(kb * QB, (kb + 1) * QB)
                enc_t = stream.tile([P, DT, QB], BF16, name="enc_t", tag="enc")
                nc.sync.dma_start(out=enc_t, in_=ek[kb])
                for pi in range(3):
                    ps = ps_mm.tile([P, QB], F32, tag="mm", name="ps_kv")
                    for t in range(DT):
                        nc.tensor.matmul(
                            ps, wkv_sb[:, t, pi * P : (pi + 1) * P],
                            enc_t[:, t, :], start=(t == 0), stop=(t == DT - 1),
                        )
                    if pi == 0:
                        nc.vector.tensor_scalar_add(
                            out=kT[0][0:DK, ks], in0=ps[0:DK], scalar1=bkA[0:DK]
                        )
                        nc.vector.tensor_scalar_add(
                            out=kT[1][DK:P, ks], in0=ps[DK:P], scalar1=bkA[DK:P]
                        )
                        nc.vector.tensor_copy(out=kT[0][DK:P, ks], in_=kT[0][0:DK, ks])
                        nc.vector.tensor_copy(out=kT[1][0:DK, ks], in_=kT[1][DK:P, ks])
                    elif pi == 1:
                        nc.vector.tensor_scalar_add(
                            out=kT[2][0:DK, ks], in0=ps[0:DK], scalar1=bkB[0:DK]
                        )
                        nc.vector.tensor_copy(out=kT[2][DK:P, ks], in_=kT[2][0:DK, ks])
                        nc.vector.tensor_copy(out=vt[0][DK:P, ks], in_=ps[DK:P])
                    else:
                        nc.vector.tensor_copy(out=vt[1][0:DK, ks], in_=ps[0:DK])
                        nc.vector.tensor_copy(out=vt[2][DK:P, ks], in_=ps[DK:P])

            # ---- V: bf16 DMA transpose into natural layout ----------------
            for h in range(HPC):
                src_rows = slice(DK, P) if h != 1 else slice(0, DK)
                nc.sync.dma_start(
                    out=v[h][:, :, 0:DK], in_=vt[h][src_rows, :], transpose=True
                )

            for _qb in range(1, QBS):
                emit_qproj(_qb)

            # ---- attention + output projection ----------------------------
            # Software pipeline across (qb, h) units: emit scores+exp for unit
            # i, then AV+normalize for unit i-1 (whose exp overlapped unit i's
            # scores on ACT), then the output projection for a q block once its
            # last head is normalized.
            NGRP = (KT_N + kgroup - 1) // kgroup
            units = [(qb, h) for qb in range(QBS) for h in range(HPC)]
            oT = {}

            def emit_scores_exp(qb, h):
                qs = slice(qb * QB, (qb + 1) * QB)
                exp_bufs = []
                for g in range(NGRP):
                    gsz = min(kgroup, KT_N - g * kgroup)
                    ps = ps_s.tile([P, kgroup * QB], F32, tag="s", name="ps_sc")
                    for j in range(gsz):
                        kt = g * kgroup + j
                        half = slice(0, DK) if kt % 2 == 0 else slice(DK, P)
                        tp = (0, 0) if kt % 2 == 0 else (DK, 0)
                        nc.tensor.matmul(
                            ps[:, j * QB : (j + 1) * QB],
                            kT[h][half, kt * P : (kt + 1) * P],
                            qT[h][half, qs],
                            start=True,
                            stop=True,
                            tile_position=tp,
                        )
                    if (g * kgroup) % EXP_TILES == 0:
                        eb = exp_pool.tile([P, EXP_TILES * QB], BF16, tag="e", name="expT")
                        exp_bufs.append(eb)
                    off = (g * kgroup) % EXP_TILES
                    nc.scalar.activation(
                        out=exp_bufs[-1][:, off * QB : (off + gsz) * QB],
                        in_=ps[:, 0 : gsz * QB],
                        func=mybir.ActivationFunctionType.Exp,
                        scale=0.125,
                    )
                return exp_bufs

            def emit_av_norm(qb, h, exp_bufs):
                av = ps_mm.tile([P, QB], F32, tag="mm", name="ps_av_t")
                for kt in range(KT_N):
                    eb = exp_bufs[kt // EXP_TILES]
                    off = kt % EXP_TILES
                    nc.tensor.matmul(
                        av[0 : DK + 1],
                        v[h][:, kt, 0 : DK + 1],
                        eb[:, off * QB : (off + 1) * QB],
                        start=(kt == 0),
                        stop=(kt == KT_N - 1),
                    )
                rt = small.tile([P, QB], F32R, tag="rt", name="recip_t")
                with nc.allow_low_precision(reason="f32r recip feeds f32r matmul"):
                    nc.vector.reciprocal(out=rt[DK : DK + 1], in_=av[DK : DK + 1])
                # broadcast the reciprocal row across partitions 0..63 with a
                # K=1 PE matmul against a ones column (row group 64)
                rb = ps_mm.tile([P, QB], F32, tag="mmo", name="ps_rb")
                nc.tensor.matmul(
                    rb[0:DK],
                    ones1[DK : DK + 1, :],
                    rt[DK : DK + 1, :],
                    start=True,
                    stop=True,
                    tile_position=(DK, 0),
                )
                rbs = small.tile([DK, QB], F32, tag="rbs", name="rb_sb")
                nc.vector.tensor_copy(out=rbs, in_=rb[0:DK])
                # heads 0,1 pack into one [128, QB] tile for a K=128 out-proj
                if h < 2:
                    if h == 0:
                        o01 = small.tile([P, QB], BF16, tag="o01", name="o01_t")
                        oT[(qb, "01")] = o01
                    dst = oT[(qb, "01")][h * DK : (h + 1) * DK, :]
                else:
                    o2 = small.tile([DK, QB], BF16, tag="o2", name="o2_t")
                    oT[(qb, "2")] = o2
                    dst = o2
                nc.vector.tensor_mul(out=dst, in0=av[0:DK], in1=rbs)

            def emit_outproj(qb):
                ysb = ysb_pool.tile([P, DT, QB], BF16, tag="y", name="y_t")
                for dt_i in range(DT):
                    pso = ps_mm.tile([P, QB], F32, tag="mmo", name="ps_o")
                    nc.tensor.matmul(
                        pso,
                        woA_sb[:, dt_i * P : (dt_i + 1) * P],
                        oT[(qb, "01")],
                        start=True,
                        stop=False,
                    )
                    nc.tensor.matmul(
                        pso,
                        woB_sb[:, dt_i * P : (dt_i + 1) * P],
                        oT[(qb, "2")],
                        start=False,
                        stop=True,
                    )
                    nc.vector.tensor_copy(out=ysb[:, dt_i, :], in_=pso)
                nc.sync.dma_start(out=yT[qb], in_=ysb)

            pend = None  # (qb, h, exp_bufs) of the previous unit
            for qb, h in units:
                ebs = emit_scores_exp(qb, h)
                if pend is not None:
                    pqb, ph, pebs = pend
                    emit_av_norm(pqb, ph, pebs)
                    if ph == HPC - 1:
                        emit_outproj(pqb)
                pend = (qb, h, ebs)
            pqb, ph, pebs = pend
            emit_av_norm(pqb, ph, pebs)
            emit_outproj(pqb)

    nc.compile()
    return nc


def _tile_T(mat_T, nblk):
    """[D, N] f32 -> [nblk, 128, DT, 512] bf16 (pre-tiled for SBUF)."""
    Dd, N = mat_T.shape
    a = mat_T.reshape(DT, P, nblk, QB).transpose(2, 1, 0, 3)
    return np.ascontiguousarray(a.astype(ml_dtypes.bfloat16))


def shard_inputs(x, encoding, w_q, b_q, w_k, b_k, w_v, b_v, w_o, b_o):
    """Full inputs -> list of 8 per-core input dicts (numpy, contiguous)."""
    N = x.shape[1]
    xT_full = np.asarray(x, np.float32)[0].T                      # [D, N]
    encT = np.asarray(encoding, np.float32)[0].T                  # [D, N]
    w_q, w_k, w_v, w_o = (np.asarray(a, np.float32) for a in (w_q, w_k, w_v, w_o))
    b_q, b_k = np.asarray(b_q, np.float32), np.asarray(b_k, np.float32)
    ek_t = _tile_T(encT, N // QB)
    in_maps = []
    for core in range(N_CORES):
        p = core // 2
        hsel = slice(HPC * p * DK, HPC * (p + 1) * DK)
        qsel = slice(0, N // 2) if core % 2 == 0 else slice(N // 2, N)
        wo_h = w_o[hsel, :]
        in_maps.append(
            {
                "xq": _tile_T(xT_full[:, qsel], (N // 2) // QB),
                "ek": ek_t,
                "wkv_d": np.ascontiguousarray(
                    np.concatenate([w_k[:, hsel], w_v[:, hsel]], axis=1)
                    .reshape(DT, P, 2 * HPC * DK)
                    .astype(ml_dtypes.bfloat16)
                    .transpose(1, 0, 2)
                ),
                "wq_d": np.ascontiguousarray(
                    w_q[:, hsel]
                    .reshape(DT, P, HPC * DK)
                    .astype(ml_dtypes.bfloat16)
                    .transpose(1, 0, 2)
                ),
                "woA_d": np.ascontiguousarray(wo_h[0:P].astype(ml_dtypes.bfloat16)),
                "woB_d": np.ascontiguousarray(wo_h[P:].astype(ml_dtypes.bfloat16)),
                "bq": np.ascontiguousarray(b_q[hsel].reshape(-1, 1)),
                "bk": np.ascontiguousarray(b_k[hsel].reshape(-1, 1)),
            }
        )
    return in_maps


def combine_outputs(results, b_v, w_o, b_o, N, dtype):
    """Per-core yT partials -> full [1, N, D] output (host-side biases)."""
    half = N // 2
    nblk = half // QB
    y = np.zeros((N, D), np.float32)
    for core, res in enumerate(results):
        yt = np.asarray(res["yT"]).astype(np.float32)  # [nblk, 128, DT, 512]
        # [nblk, P, DT, QB] -> [D, half]: d = dt*P + p, n = qb*QB + q
        part = yt.transpose(2, 1, 0, 3).reshape(D, half).T
        if core % 2 == 0:
            y[:half] += part
        else:
            y[half:] += part
    y += np.asarray(b_v, np.float32) @ np.asarray(w_o, np.float32) + np.asarray(
        b_o, np.float32
    )
    return np.ascontiguousarray(y[None]).astype(dtype)


_PROGRAM_CACHE = {}


def _get_program():
    key = "main"
    if key not in _PROGRAM_CACHE:
        _PROGRAM_CACHE[key] = build_program()
    return _PROGRAM_CACHE[key]


def kernel(x, encoding, w_q, b_q, w_k, b_k, w_v, b_v, w_o, b_o):
    nc = _get_program()
    in_maps = shard_inputs(x, encoding, w_q, b_q, w_k, b_k, w_v, b_v, w_o, b_o)
    res = run_bass_kernel_spmd(nc, in_maps, core_ids=list(range(N_CORES)))
    return combine_outputs(
        res.results, b_v, w_o, b_o, np.asarray(x).shape[1], np.asarray(x).dtype
    )


# revision 18
# speedup vs baseline: 1.0655x; 1.0256x over previous
"""Trainium2 Bass kernel: multi-head encoder-decoder attention.

nn_MultiHeadEncDecAttention — B=1, N=4096, d_model=768, 12 heads, d_k=64.

Self-contained harness entry point: `kernel(**inputs) -> np.ndarray` takes
the FULL unsharded inputs (as produced by the problem's setup_inputs()),
distributes the work across 8 NeuronCores (heads x query-range sharding),
runs a Bass/Tile SPMD program, and reassembles the full output.

Sharding: core pair p in {0..3} owns heads {3p, 3p+1, 3p+2}; within a
pair, core 2p handles query rows [0, 2048) and core 2p+1 rows [2048, 4096).
Each core computes its heads' attention for its query rows plus the partial
output projection for those heads; the host sums the 4 head-partials per
query half (b_o and the b_v contribution are folded in on the host, which
is exact because softmax rows sum to 1).

Optimizations vs the original baseline (measured ~2x faster device body):
  - all tensors bf16 (host-converted), halving HBM traffic
  - host pre-tiles every DRAM operand into SBUF layout so each DMA is one
    contiguous run per partition (~128 descriptors instead of ~768), and the
    SP DMA queue carries only ~27 transfers total (no SBUF-SBUF copies)
  - SBUF half-duplications done as DVE partition-shifted copies instead of
    DMAs (keeps the serial DMA dispatch queue free for input streaming)
  - output projection packs heads 0+1 into one 128-contraction matmul
    (w_o rows 0:128 as one stationary tile)
  - yT returned as bf16 in store-friendly tiling, host accumulates in f32
  - exp pool 6 buffers to decouple ACT exp production from PE consumption
  (gpsimd/SWDGE deliberately unused: any gpsimd instruction adds ~110us of
  fixed per-execution runtime cost; prologue interleaving of q-block-0
  scores into the K/V projection was tried and measured slower on HW)
"""

import sys

sys.path.insert(0, "/opt/trn_rl_repo")

from contextlib import ExitStack

import numpy as np
import ml_dtypes

import concourse.tile as tile
from concourse import bacc, mybir
from concourse.bass_utils import run_bass_kernel_spmd

F32 = mybir.dt.float32
F32R = mybir.dt.float32r
BF16 = mybir.dt.bfloat16

D = 768          # d_model
DK = 64          # per-head dim
HPC = 3          # heads per core
P = 128          # SBUF partitions
QB = 512         # matmul moving-dim block
DT = D // P      # contraction k-tiles over d_model
N_CORES = 8


def build_program(NQ=2048, NK=4096, kgroup=2, repeat=1):
    """Build + compile the per-core SPMD program (identical on all cores).

    Layout strategy (everything "transposed"; host passes x^T / enc^T tiled):
      QT[h] [128, NQ], KT[h] [128, NK] — data in both partition halves so
                                         scores matmuls can PE-row-tile
      V[h]  [128, 32, 96]              — natural layout via bf16 DMA
                                         transpose; column 64 = ones
      scoresT = KT-tile.T @ QT-block  -> PSUM [128(kpos), 512(q)]
      expT    = exp(0.125*scoresT)    -> SBUF bf16 (ACT, scale folded in)
      AV      = [V|1].T @ expT        -> PSUM [65, 512]; row 64 = denom
      yT     += w_o-slice.T @ (AV[0:64] * recip(AV[64]))  over heads
    """
    KT_N = NK // P           # kpos tiles
    QBS = NQ // QB           # q blocks
    KB_N = NK // QB          # kpos blocks for the K/V projection
    EXP_TILES = 12 if KT_N >= 12 else KT_N   # kpos-tiles per exp buffer

    nc = bacc.Bacc("TRN2", target_bir_lowering=False, debug=False)

    # host-pretiled DRAM operands: one contiguous run per partition line
    xq = nc.dram_tensor("xq", [QBS, P, DT, QB], BF16, kind="ExternalInput").ap()
    ek = nc.dram_tensor("ek", [KB_N, P, DT, QB], BF16, kind="ExternalInput").ap()
    wq_d = nc.dram_tensor("wq_d", [P, DT, HPC * DK], BF16, kind="ExternalInput").ap()
    # wkv = concat([w_k cols, w_v cols]); projection passes use 128-col
    # slices: [wk0|wk1], [wk2|wv0], [wv1|wv2]
    wkv_d = nc.dram_tensor("wkv_d", [P, DT, 2 * HPC * DK], BF16, kind="ExternalInput").ap()
    woA_d = nc.dram_tensor("woA_d", [P, D], BF16, kind="ExternalInput").ap()  # heads 0,1
    woB_d = nc.dram_tensor("woB_d", [DK, D], BF16, kind="ExternalInput").ap()  # head 2
    bq = nc.dram_tensor("bq", [HPC * DK, 1], F32, kind="ExternalInput").ap()
    bk = nc.dram_tensor("bk", [HPC * DK, 1], F32, kind="ExternalInput").ap()
    yT = nc.dram_tensor("yT", [QBS, P, DT, QB], BF16, kind="ExternalOutput").ap()

    with tile.TileContext(nc) as tc, ExitStack() as ctx:
        consts = ctx.enter_context(tc.tile_pool(name="consts", bufs=1))
        persist = ctx.enter_context(tc.tile_pool(name="persist", bufs=1))
        stream = ctx.enter_context(tc.tile_pool(name="stream", bufs=3))
        small = ctx.enter_context(tc.tile_pool(name="small", bufs=2))
        ysb_pool = ctx.enter_context(tc.tile_pool(name="ysb", bufs=2))
        exp_pool = ctx.enter_context(tc.tile_pool(name="exp", bufs=6))
        ps_s = ctx.enter_context(tc.tile_pool(name="ps_s", bufs=2, space="PSUM"))
        ps_mm = ctx.enter_context(tc.tile_pool(name="ps_mm", bufs=2, space="PSUM"))

        for _rep in range(repeat):
            # ---- constants -------------------------------------------------
            wkv_sb = consts.tile([P, DT, 2 * HPC * DK], BF16)
            nc.sync.dma_start(out=wkv_sb, in_=wkv_d)
            wq_sb = consts.tile([P, DT, HPC * DK], BF16)
            nc.sync.dma_start(out=wq_sb, in_=wq_d)
            woA_sb = consts.tile([P, D], BF16)
            nc.sync.dma_start(out=woA_sb, in_=woA_d)
            woB_sb = consts.tile([DK, D], BF16)
            nc.sync.dma_start(out=woB_sb, in_=woB_d)
            bqA = consts.tile([P, 1], F32)
            nc.sync.dma_start(out=bqA, in_=bq[0:P, :])
            bqB = consts.tile([DK, 1], F32)
            nc.sync.dma_start(out=bqB, in_=bq[P : P + DK, :])
            bkA = consts.tile([P, 1], F32)
            nc.sync.dma_start(out=bkA, in_=bk[0:P, :])
            bkB = consts.tile([DK, 1], F32)
            nc.sync.dma_start(out=bkB, in_=bk[P : P + DK, :])
            ones1_f = consts.tile([P, DK], F32)
            nc.vector.memset(ones1_f[DK : DK + 1, :], 1.0)
            ones1 = ones1_f.bitcast(F32R)

            # ---- persistent per-head tensors ------------------------------
            kT = [persist.tile([P, NK], BF16, name=f"kT{h}") for h in range(HPC)]
            qT = [persist.tile([P, NQ], BF16, name=f"qT{h}") for h in range(HPC)]
            # V row stride padded to 96 els (192 B): keeps each DMA-transpose
            # dest 32-byte aligned (XBAR); col 64 = ones, cols 65..95 zero
            v = [persist.tile([P, KT_N, 96], BF16, name=f"v{h}") for h in range(HPC)]
            # vt shares the exp pool tag: dead after the V transposes, so its
            # slots are recycled as exp buffers during attention
            vt = [exp_pool.tile([P, NK], BF16, name=f"vt{h}", tag="e") for h in range(HPC)]
            for h in range(HPC):
                nc.vector.memset(v[h][:, :, DK : DK + 1], 1.0)
                nc.vector.memset(v[h][:, :, DK + 1 : 96], 0.0)

            # ---- Q projection (DVE bias-adds + partition-shifted dups) ----
            def emit_qproj(qb):
                qs = slice(qb * QB, (qb + 1) * QB)
                x_t = stream.tile([P, DT, QB], BF16, name="x_t", tag="enc")
                nc.sync.dma_start(out=x_t, in_=xq[qb])
                ps = ps_mm.tile([P, QB], F32, tag="mm", name="ps_q01")
                for t in range(DT):
                    nc.tensor.matmul(
                        ps, wq_sb[:, t, 0:P], x_t[:, t, :],
                        start=(t == 0), stop=(t == DT - 1),
                    )
                nc.vector.tensor_scalar_add(
                    out=qT[0][0:DK, qs], in0=ps[0:DK], scalar1=bqA[0:DK]
                )
                nc.vector.tensor_scalar_add(
                    out=qT[1][DK:P, qs], in0=ps[DK:P], scalar1=bqA[DK:P]
                )
                ps2 = ps_mm.tile([P, QB], F32, tag="mm", name="ps_q2")
                for t in range(DT):
                    nc.tensor.matmul(
                        ps2[0:DK], wq_sb[:, t, P : P + DK], x_t[:, t, :],
                        start=(t == 0), stop=(t == DT - 1),
                    )
                nc.vector.tensor_scalar_add(
                    out=qT[2][0:DK, qs], in0=ps2[0:DK], scalar1=bqB[0:DK]
                )
                nc.vector.tensor_copy(out=qT[0][DK:P, qs], in_=qT[0][0:DK, qs])
                nc.vector.tensor_copy(out=qT[1][0:DK, qs], in_=qT[1][DK:P, qs])
                nc.vector.tensor_copy(out=qT[2][DK:P, qs], in_=qT[2][0:DK, qs])

            emit_qproj(0)

            # ---- K/V projection -------------------------------------------
            for kb in range(KB_N):
                ks = slice(kb * QB, (kb + 1) * QB)
                enc_t = stream.tile([P, DT, QB], BF16, name="enc_t", tag="enc")
                nc.sync.dma_start(out=enc_t, in_=ek[kb])
                for pi in range(3):
                    ps = ps_mm.tile([P, QB], F32, tag="mm", name="ps_kv")
                    for t in range(DT):
                        nc.tensor.matmul(
                            ps, wkv_sb[:, t, pi * P : (pi + 1) * P],
                            enc_t[:, t, :], start=(t == 0), stop=(t == DT - 1),
                        )
                    if pi == 0:
                        nc.vector.tensor_scalar_add(
                            out=kT[0][0:DK, ks], in0=ps[0:DK], scalar1=bkA[0:DK]
                        )
                        nc.vector.tensor_scalar_add(
                            out=kT[1][DK:P, ks], in0=ps[DK:P], scalar1=bkA[DK:P]
                        )
                        nc.vector.tensor_copy(out=kT[0][DK:P, ks], in_=kT[0][0:DK, ks])
                        nc.vector.tensor_copy(out=kT[1][0:DK, ks], in_=kT[1][DK:P, ks])
                    elif pi == 1:
                        nc.vector.tensor_scalar_add(
                            out=kT[2][0:DK, ks], in0=ps[0:DK], scalar1=bkB[0:DK]
                        )
                        nc.vector.tensor_copy(out=kT[2][DK:P, ks], in_=kT[2][0:DK, ks])
                        nc.vector.tensor_copy(out=vt[0][DK:P, ks], in_=ps[DK:P])
                    else:
                        nc.vector.tensor_copy(out=vt[1][0:DK, ks], in_=ps[0:DK])
                        nc.vector.tensor_copy(out=vt[2][DK:P, ks], in_=ps[DK:P])

            # ---- V: bf16 DMA transpose into natural layout ----------------
            for h in range(HPC):
                src_rows = slice(DK, P) if h != 1 else slice(0, DK)
                nc.sync.dma_start(
                    out=v[h][:, :, 0:DK], in_=vt[h][src_rows, :], transpose=True
                )

            for _qb in range(1, QBS):
                emit_qproj(_qb)

            # ---- attention + output projection ----------------------------
            # Software pipeline across (qb, h) units: emit scores+exp for unit
            # i, then AV+normalize for unit i-1 (whose exp overlapped unit i's
            # scores on ACT), then the output projection for a q block once its
            # last head is normalized.
            NGRP = (KT_N + kgroup - 1) // kgroup
            units = [(qb, h) for qb in range(QBS) for h in range(HPC)]
            oT = {}

            def emit_scores_exp(qb, h):
                qs = slice(qb * QB, (qb + 1) * QB)
                exp_bufs = []
                for g in range(NGRP):
                    gsz = min(kgroup, KT_N - g * kgroup)
                    ps = ps_s.tile([P, kgroup * QB], F32, tag="s", name="ps_sc")
                    for j in range(gsz):
                        kt = g * kgroup + j
                        half = slice(0, DK) if kt % 2 == 0 else slice(DK, P)
                        tp = (0, 0) if kt % 2 == 0 else (DK, 0)
                        nc.tensor.matmul(
                            ps[:, j * QB : (j + 1) * QB],
                            kT[h][half, kt * P : (kt + 1) * P],
                            qT[h][half, qs],
                            start=True,
                            stop=True,
                            tile_position=tp,
                        )
                    if (g * kgroup) % EXP_TILES == 0:
                        eb = exp_pool.tile([P, EXP_TILES * QB], BF16, tag="e", name="expT")
                        exp_bufs.append(eb)
                    off = (g * kgroup) % EXP_TILES
                    nc.scalar.activation(
                        out=exp_bufs[-1][:, off * QB : (off + gsz) * QB],
                        in_=ps[:, 0 : gsz * QB],
                        func=mybir.ActivationFunctionType.Exp,
                        scale=0.125,
                    )
                return exp_bufs

            def emit_av_norm(qb, h, exp_bufs):
                av = ps_mm.tile([P, QB], F32, tag="mm", name="ps_av_t")
                for kt in range(KT_N):
                    eb = exp_bufs[kt // EXP_TILES]
                    off = kt % EXP_TILES
                    nc.tensor.matmul(
                        av[0 : DK + 1],
                        v[h][:, kt, 0 : DK + 1],
                        eb[:, off * QB : (off + 1) * QB],
                        start=(kt == 0),
                        stop=(kt == KT_N - 1),
                    )
                rt = small.tile([P, QB], F32R, tag="rt", name="recip_t")
                with nc.allow_low_precision(reason="f32r recip feeds f32r matmul"):
                    nc.vector.reciprocal(out=rt[DK : DK + 1], in_=av[DK : DK + 1])
                # broadcast the reciprocal row across partitions 0..63 with a
                # K=1 PE matmul against a ones column (row group 64)
                rb = ps_mm.tile([P, QB], F32, tag="mmo", name="ps_rb")
                nc.tensor.matmul(
                    rb[0:DK],
                    ones1[DK : DK + 1, :],
                    rt[DK : DK + 1, :],
                    start=True,
                    stop=True,
                    tile_position=(DK, 0),
                )
                rbs = small.tile([DK, QB], F32, tag="rbs", name="rb_sb")
                nc.vector.tensor_copy(out=rbs, in_=rb[0:DK])
                # heads 0,1 pack into one [128, QB] tile for a K=128 out-proj
                if h < 2:
                    if h == 0:
                        o01 = small.tile([P, QB], BF16, tag="o01", name="o01_t")
                        oT[(qb, "01")] = o01
                    dst = oT[(qb, "01")][h * DK : (h + 1) * DK, :]
                else:
                    o2 = small.tile([DK, QB], BF16, tag="o2", name="o2_t")
                    oT[(qb, "2")] = o2
                    dst = o2
                nc.vector.tensor_mul(out=dst, in0=av[0:DK], in1=rbs)

            def emit_outproj(qb):
                ysb = ysb_pool.tile([P, DT, QB], BF16, tag="y", name="y_t")
                for dt_i in range(DT):
                    pso = ps_mm.tile([P, QB], F32, tag="mmo", name="ps_o")
                    nc.tensor.matmul(
                        pso,
                        woA_sb[:, dt_i * P : (dt_i + 1) * P],
                        oT[(qb, "01")],
                        start=True,
                        stop=False,
                    )
                    nc.tensor.matmul(
                        pso,
                        woB_sb[:, dt_i * P : (dt_i + 1) * P],
                        oT[(qb, "2")],
                        start=False,
                        stop=True,
                    )
                    nc.vector.tensor_copy(out=ysb[:, dt_i, :], in_=pso)
                nc.sync.dma_start(out=yT[qb], in_=ysb)

            pend = None  # (qb, h, exp_bufs) of the previous unit
            for qb, h in units:
                ebs = emit_scores_exp(qb, h)
                if pend is not None:
                    pqb, ph, pebs = pend
                    emit_av_norm(pqb, ph, pebs)
                    if ph == HPC - 1:
                        emit_outproj(pqb)
                pend = (qb, h, ebs)
            pqb, ph, pebs = pend
            emit_av_norm(pqb, ph, pebs)
            emit_outproj(pqb)

    nc.compile()
    return nc


def _tile_T(mat_T, nblk):
    """[D, N] f32 -> [nblk, 128, DT, 512] bf16 (pre-tiled for SBUF)."""
    Dd, N = mat_T.shape
    a = mat_T.reshape(DT, P, nblk, QB).transpose(2, 1, 0, 3)
    return np.ascontiguousarray(a.astype(ml_dtypes.bfloat16))


def shard_inputs(x, encoding, w_q, b_q, w_k, b_k, w_v, b_v, w_o, b_o):
    """Full inputs -> list of 8 per-core input dicts (numpy, contiguous)."""
    N = x.shape[1]
    xT_full = np.asarray(x, np.float32)[0].T                      # [D, N]
    encT = np.asarray(encoding, np.float32)[0].T                  # [D, N]
    w_q, w_k, w_v, w_o = (np.asarray(a, np.float32) for a in (w_q, w_k, w_v, w_o))
    b_q, b_k = np.asarray(b_q, np.float32), np.asarray(b_k, np.float32)
    ek_t = _tile_T(encT, N // QB)
    in_maps = []
    for core in range(N_CORES):
        p = core // 2
        hsel = slice(HPC * p * DK, HPC * (p + 1) * DK)
        qsel = slice(0, N // 2) if core % 2 == 0 else slice(N // 2, N)
        wo_h = w_o[hsel, :]
        in_maps.append(
            {
                "xq": _tile_T(xT_full[:, qsel], (N // 2) // QB),
                "ek": ek_t,
                "wkv_d": np.ascontiguousarray(
                    np.concatenate([w_k[:, hsel], w_v[:, hsel]], axis=1)
                    .reshape(DT, P, 2 * HPC * DK)
                    .astype(ml_dtypes.bfloat16)
                    .transpose(1, 0, 2)
                ),
                "wq_d": np.ascontiguousarray(
                    w_q[:, hsel]
                    .reshape(DT, P, HPC * DK)
                    .astype(ml_dtypes.bfloat16)
                    .transpose(1, 0, 2)
                ),
                "woA_d": np.ascontiguousarray(wo_h[0:P].astype(ml_dtypes.bfloat16)),
                "woB_d": np.ascontiguousarray(wo_h[P:].astype(ml_dtypes.bfloat16)),
                "bq": np.ascontiguousarray(b_q[hsel].reshape(-1, 1)),
                "bk": np.ascontiguousarray(b_k[hsel].reshape(-1, 1)),
            }
        )
    return in_maps


def combine_outputs(results, b_v, w_o, b_o, N, dtype):
    """Per-core yT partials -> full [1, N, D] output (host-side biases)."""
    half = N // 2
    nblk = half // QB
    y = np.zeros((N, D), np.float32)
    for core, res in enumerate(results):
        yt = np.asarray(res["yT"]).astype(np.float32)  # [nblk, 128, DT, 512]
        # [nblk, P, DT, QB] -> [D, half]: d = dt*P + p, n = qb*QB + q
        part = yt.transpose(2, 1, 0, 3).reshape(D, half).T
        if core % 2 == 0:
            y[:half] += part
        else:
            y[half:] += part
    y += np.asarray(b_v, np.float32) @ np.asarray(w_o, np.float32) + np.asarray(
        b_o, np.float32
    )
    return np.ascontiguousarray(y[None]).astype(dtype)


_PROGRAM_CACHE = {}


def _get_program():
    key = "main"
    if key not in _PROGRAM_CACHE:
        _PROGRAM_CACHE[key] = build_program()
    return _PROGRAM_CACHE[key]


def kernel(x, encoding, w_q, b_q, w_k, b_k, w_v, b_v, w_o, b_o):
    nc = _get_program()
    in_maps = shard_inputs(x, encoding, w_q, b_q, w_k, b_k, w_v, b_v, w_o, b_o)
    res = run_bass_kernel_spmd(nc, in_maps, core_ids=list(range(N_CORES)))
    return combine_outputs(
        res.results, b_v, w_o, b_o, np.asarray(x).shape[1], np.asarray(x).dtype
    )
